# revision 17
# baseline (speedup 1.0000x reference)
"""Trainium2 Bass kernel for nn_CustomMultiHeadAttention_20418274525443.

Self-contained: takes FULL unsharded inputs (as produced by the problem's
setup_inputs), shards across 8 NeuronCores, runs a Bass/Tile kernel via
run_bass_kernel_spmd, and gathers the full output.

Sharding: core c handles batch b = c//4 and heads 4*(c%4) .. 4*(c%4)+3
(data parallel on B x tensor parallel on heads). Each core computes its
partial output projection (contribution of its 256 hidden dims); the host
sums the 4 partials per batch and adds the output bias.

Math per core (Dh = 64, scale = 1/8):
  Q^T = (Wq_slice @ x_q^T) + bq    [256, 2048]  (d-on-partitions layout)
  K^T likewise; V = x_v @ Wv_slice^T + bv  [2048, 256] (natural layout)
  per head h: scores^T[k,q] = K_h^T.T @ Q_h^T  (PSUM, preloaded with M')
    M'[k,q] = 8*alpha*copysign(log1p|f_q - f_k|, f_q - f_k) (+8*attn_mask^T)
    probs^T = Exp(0.125 * PSUM + kpm_bias_k)  -> bf16
    outext^T[dv+1, q] = [V_h | 1]^T.T-style: lhsT=[V_h|ones] so row 64 = sum_k probs
    scaled^T = outext^T[0:64] * (1 / outext^T[64])   (per-q denominators)
  out_partial[q, :] = concat_h(scaled^T).T @ Wo_slice^T  [2048, 1024] fp32
"""
import sys

sys.path.insert(0, "/opt/trn_rl_repo")

import numpy as np
import ml_dtypes

import concourse.bass as bass
import concourse.tile as tile
from concourse import bacc, mybir
from concourse.bass_utils import run_bass_kernel_spmd
from concourse.masks import make_identity

D_MODEL = 1024
NHEAD = 16
HEAD_DIM = 64
B, T = 2, 2048
N_CORES = 8
HPC = 4               # heads per core
DSL = HPC * HEAD_DIM  # 256 = per-core slice of d_model
SCALE = HEAD_DIM ** -0.5  # 0.125
QH = 2                # q halves
QHW = T // QH         # 1024
KT = T // 128         # 16 k tiles
F32 = mybir.dt.float32
F16 = mybir.dt.float16
BF16 = mybir.dt.bfloat16
AF = mybir.ActivationFunctionType
ALU = mybir.AluOpType

_PROGRAM_CACHE = {}


def _build_program(use_attn_mask: bool):
    nc = bacc.Bacc(num_devices=N_CORES)

    # ---- per-core DRAM inputs (host pre-sliced / transposed / cast) ----
    xq_t = nc.dram_tensor("xq_t", [D_MODEL, T], BF16, kind="ExternalInput")
    xk_t = nc.dram_tensor("xk_t", [D_MODEL, T], BF16, kind="ExternalInput")
    xv_t = nc.dram_tensor("xv_t", [D_MODEL, T], BF16, kind="ExternalInput")
    wq_t = nc.dram_tensor("wq_t", [D_MODEL, DSL], BF16, kind="ExternalInput")
    wk_t = nc.dram_tensor("wk_t", [D_MODEL, DSL], BF16, kind="ExternalInput")
    wv_t = nc.dram_tensor("wv_t", [D_MODEL, DSL], BF16, kind="ExternalInput")
    wo_t = nc.dram_tensor("wo_t", [DSL, D_MODEL], BF16, kind="ExternalInput")
    bq_d = nc.dram_tensor("bq_d", [DSL], F32, kind="ExternalInput")
    bk_d = nc.dram_tensor("bk_d", [DSL], F32, kind="ExternalInput")
    bv_d = nc.dram_tensor("bv_d", [DSL], F32, kind="ExternalInput")
    f_d = nc.dram_tensor("f_d", [T], F32, kind="ExternalInput")
    kpm_d = nc.dram_tensor("kpm_d", [T], F32, kind="ExternalInput")
    alpha_d = nc.dram_tensor("alpha_d", [2], F32, kind="ExternalInput")
    if use_attn_mask:
        am_d = nc.dram_tensor("am_d", [T, T], F16, kind="ExternalInput")
    out_d = nc.dram_tensor("out_d", [T, D_MODEL], F32, kind="ExternalOutput")

    def bcast_ap(src_ap, parts):
        # partition-broadcast of a 1-row / 1-d tensor across `parts` partitions
        return bass.AP(tensor=src_ap.tensor, offset=src_ap.offset,
                       ap=[[0, parts]] + list(src_ap.ap[-1:]))

    def mm(out_ap, lhsT, rhs, start, stop, nmax=512):
        # matmul with the free dim chunked to one PSUM bank (<=512 fp32)
        n = rhs.shape[-1]
        assert out_ap.shape[-1] == n
        for c0 in range(0, n, nmax):
            c = slice(c0, min(c0 + nmax, n))
            nc.tensor.matmul(out_ap[:, c], lhsT, rhs[:, c],
                             start=start, stop=stop)

    with tile.TileContext(nc) as tc:
        import contextlib
        with contextlib.ExitStack() as ctx:
            # NOTE: pool `bufs` is per-tag; tiles with distinct names/tags each
            # get their own `bufs` slots.
            const = ctx.enter_context(tc.tile_pool(name="const", bufs=1))
            qk_pool = ctx.enter_context(tc.tile_pool(name="qk", bufs=1))
            v_pool = ctx.enter_context(tc.tile_pool(name="vsb", bufs=1))
            w2_pool = ctx.enter_context(tc.tile_pool(name="wo", bufs=1))
            mp_pool = ctx.enter_context(tc.tile_pool(name="mp", bufs=KT))
            tmp_pool = ctx.enter_context(tc.tile_pool(name="tmp", bufs=3))
            probs_pool = ctx.enter_context(tc.tile_pool(name="probs", bufs=3))
            den_pool = ctx.enter_context(tc.tile_pool(name="den", bufs=2))
            opair_pool = ctx.enter_context(tc.tile_pool(name="opair", bufs=4))
            ostage_pool = ctx.enter_context(tc.tile_pool(name="ostage", bufs=3))

            # ---- constants ----
            # identity scaled by 8*alpha: the M'-preload matmul I'.T @ M'
            # then contributes 8*alpha*copysign(log1p|df|, df) to the scores
            # PSUM, so M' itself stays a pure +-log1p and alpha stays a
            # runtime input (no recompile on alpha change).
            ident_f = const.tile([128, 128], F32)
            make_identity(nc, ident_f[:, :])
            ident = const.tile([128, 128], F16)
            ones64 = const.tile([1, 64], F32)
            nc.vector.memset(ones64[:, :], 1.0)

            fq_bc = const.tile([128, T], F32)
            nc.gpsimd.dma_start(out=fq_bc[:, :], in_=bcast_ap(f_d.ap(), 128))
            fk_col = const.tile([128, KT], F32)
            nc.sync.dma_start(out=fk_col[:, :],
                              in_=f_d.ap().rearrange("(t p) -> p t", p=128))
            kpm_col = const.tile([128, KT], F32)
            nc.sync.dma_start(out=kpm_col[:, :],
                              in_=kpm_d.ap().rearrange("(t p) -> p t", p=128))
            kbias = const.tile([128, KT], F32)
            nc.vector.tensor_scalar_mul(kbias[:, :], kpm_col[:, :], -30000.0)
            al_col = const.tile([128, 2], F32)
            nc.gpsimd.dma_start(out=al_col[:, :], in_=bcast_ap(alpha_d.ap(), 128))
            nc.vector.tensor_scalar(ident[:, :], ident_f[:, :],
                                    al_col[:, 0:1], None, op0=ALU.mult)
            bq_col = const.tile([128, 2], F32)
            nc.sync.dma_start(out=bq_col[:, :],
                              in_=bq_d.ap().rearrange("(t p) -> p t", p=128))
            bk_col = const.tile([128, 2], F32)
            nc.sync.dma_start(out=bk_col[:, :],
                              in_=bk_d.ap().rearrange("(t p) -> p t", p=128))
            bv_bc = const.tile([128, DSL], F32)
            nc.gpsimd.dma_start(out=bv_bc[:, :], in_=bcast_ap(bv_d.ap(), 128))

            # ---- weights ----
            w_sb = {}
            for nm, dram in (("q", wq_t), ("k", wk_t), ("v", wv_t)):
                for di in range(8):
                    t_ = const.tile([128, DSL], BF16, name=f"w{nm}{di}",
                                    tag=f"w{nm}{di}")
                    nc.sync.dma_start(out=t_[:, :],
                                      in_=dram.ap()[di * 128:(di + 1) * 128, :])
                    w_sb[nm, di] = t_
            wo_sb = []
            for pr in range(2):
                t_ = w2_pool.tile([128, D_MODEL], BF16, name=f"wo{pr}")
                nc.sync.dma_start(out=t_[:, :],
                                  in_=wo_t.ap()[pr * 128:(pr + 1) * 128, :])
                wo_sb.append(t_)

            if use_attn_mask:
                am_sb = []
                for kt in range(KT):
                    t_ = const.tile([128, T], F16, name=f"am{kt}", tag=f"am{kt}")
                    # host passes 8 * attn_mask^T, so [k, q] orientation
                    nc.sync.dma_start(out=t_[:, :],
                                      in_=am_d.ap()[kt * 128:(kt + 1) * 128, :])
                    am_sb.append(t_)

            # ---- phase 1: projections ----
            qt_sb, kt_sb = [], []
            for i in range(2):
                qt_sb.append(qk_pool.tile([128, T], BF16, name=f"qt{i}"))
                kt_sb.append(qk_pool.tile([128, T], BF16, name=f"kt{i}"))
            v_sb = []
            for kt in range(KT):
                t_ = v_pool.tile([128, HPC * 65], BF16, name=f"v{kt}")
                nc.vector.memset(t_[:, :], 1.0)  # ones columns survive at 65k+64
                v_sb.append(t_)

            with tc.tile_pool(name="xt", bufs=9) as xt_pool, \
                 tc.tile_pool(name="psA", bufs=2, space="PSUM") as psA:
                for nm, xdram, bias_col, outs in (
                        ("q", xq_t, bq_col, qt_sb), ("k", xk_t, bk_col, kt_sb)):
                    x_tiles = []
                    for di in range(8):
                        xt_ = xt_pool.tile([128, T], BF16, name=f"x{nm}{di}",
                                           tag="xt")
                        nc.sync.dma_start(
                            out=xt_[:, :],
                            in_=xdram.ap()[di * 128:(di + 1) * 128, :])
                        x_tiles.append(xt_)
                    for do_t in range(2):
                        for nch in range(2):
                            ps = psA.tile([128, QHW], F32, tag="psA")
                            for di in range(8):
                                mm(ps[:, :],
                                   w_sb[nm, di][:, do_t * 128:(do_t + 1) * 128],
                                   x_tiles[di][:, nch * QHW:(nch + 1) * QHW],
                                   start=(di == 0), stop=(di == 7))
                            # PSUM -> SBUF bf16 with per-partition bias add
                            nc.vector.tensor_scalar(
                                outs[do_t][:, nch * QHW:(nch + 1) * QHW],
                                ps[:, :], bias_col[:, do_t:do_t + 1], None,
                                op0=ALU.add)
                # V projection (natural layout)
                x_tiles = []
                for di in range(8):
                    xt_ = xt_pool.tile([128, T], BF16, name=f"xv{di}", tag="xt")
                    nc.sync.dma_start(out=xt_[:, :],
                                      in_=xv_t.ap()[di * 128:(di + 1) * 128, :])
                    x_tiles.append(xt_)
                for tt in range(KT):
                    ps = psA.tile([128, DSL], F32, tag="psA")
                    for di in range(8):
                        mm(ps[:, :],
                           x_tiles[di][:, tt * 128:(tt + 1) * 128],
                           w_sb["v", di][:, :],
                           start=(di == 0), stop=(di == 7))
                    # strided copy into cols h*65..h*65+63 with bv add; the
                    # ones columns at h*65+64 remain from the memset
                    vdst = v_sb[tt][:, :].rearrange(
                        "p (h e) -> p h e", e=65)[:, :, 0:HEAD_DIM]
                    nc.vector.tensor_tensor(
                        vdst,
                        ps[:, :].rearrange("p (h e) -> p h e", e=HEAD_DIM),
                        bv_bc[:, :].rearrange("p (h e) -> p h e", e=HEAD_DIM),
                        op=ALU.add)

            # ---- phase 2: attention + output proj, per q-half ----
            # PSUM budget (8 banks): psS tag (scores / denom-bcast / O-proj
            # share 2 slots x 2 banks = 4) + psO (2 slots x 2 banks = 4).
            with tc.tile_pool(name="psS", bufs=2, space="PSUM") as psS, \
                 tc.tile_pool(name="psO", bufs=2, space="PSUM") as psO:
                for qh in range(QH):
                    qsl = slice(qh * QHW, (qh + 1) * QHW)
                    # M' tiles for this q half
                    mp_tiles = []
                    for kt in range(KT):
                        d_t = tmp_pool.tile([128, QHW], F16, name="d_t", tag="d")
                        nc.vector.tensor_scalar(
                            d_t[:, :], fq_bc[:, qsl], fk_col[:, kt:kt + 1], None,
                            op0=ALU.subtract)
                        ge_t = tmp_pool.tile([128, QHW], F16, name="ge_t",
                                             tag="ge")
                        nc.vector.tensor_scalar(
                            ge_t[:, :], d_t[:, :], 0.0, None, op0=ALU.is_ge)
                        sg_t = tmp_pool.tile([128, QHW], F16, name="sg_t",
                                             tag="sg")
                        # ge*2 - 1  ->  +-1 (alpha lives in the scaled identity)
                        nc.vector.tensor_scalar(
                            sg_t[:, :], ge_t[:, :], 2.0, -1.0,
                            op0=ALU.mult, op1=ALU.add)
                        a_t = tmp_pool.tile([128, QHW], F16, name="a_t", tag="a")
                        nc.vector.tensor_tensor(a_t[:, :], d_t[:, :], sg_t[:, :],
                                                op=ALU.mult)  # |d|
                        g_t = tmp_pool.tile([128, QHW], F16, name="g_t", tag="g")
                        nc.scalar.activation(g_t[:, :], a_t[:, :], AF.Ln,
                                             bias=1.0, scale=1.0)
                        mp = mp_pool.tile([128, QHW], F16, name="mp", tag="mp")
                        if use_attn_mask:
                            nc.vector.scalar_tensor_tensor(
                                mp[:, :], g_t[:, :], 1.0, sg_t[:, :],
                                op0=ALU.bypass, op1=ALU.mult)
                            nc.vector.tensor_tensor(
                                mp[:, :], mp[:, :], am_sb[kt][:, qsl],
                                op=ALU.add)
                        else:
                            nc.vector.tensor_tensor(mp[:, :], g_t[:, :],
                                                    sg_t[:, :], op=ALU.mult)
                        mp_tiles.append(mp)

                    opairs = []
                    for h in range(HPC):
                        pr_i = h // 2
                        hh = h % 2
                        if hh == 0:
                            op_t = opair_pool.tile([128, QHW], BF16,
                                                   name=f"opair{pr_i}",
                                                   tag="opair")
                            opairs.append(op_t)
                        ot = psO.tile([65, QHW], F32, tag="psO")
                        for kt in range(KT):
                            sc = psS.tile([128, QHW], F32, tag="psS")
                            mm(sc[:, :], ident[:, :], mp_tiles[kt][:, :],
                               start=True, stop=False)
                            mm(sc[:, :],
                               kt_sb[pr_i][hh * 64:(hh + 1) * 64,
                                           kt * 128:(kt + 1) * 128],
                               qt_sb[pr_i][hh * 64:(hh + 1) * 64, qsl],
                               start=False, stop=True)
                            pr = probs_pool.tile([128, QHW], BF16, name="pr",
                                                 tag="pr")
                            nc.scalar.activation(pr[:, :], sc[:, :], AF.Exp,
                                                 bias=kbias[:, kt:kt + 1],
                                                 scale=SCALE)
                            mm(ot[:, :], v_sb[kt][:, h * 65:(h + 1) * 65],
                               pr[:, :], start=(kt == 0), stop=(kt == KT - 1))
                        # normalize rows by the sums row (row 64): reciprocal
                        # of the PSUM row into SBUF, broadcast across 64
                        # partitions via a K=1 ones outer-product on PE
                        # (SBUF APs can't stride-0 partitions, DMA can't
                        # read PSUM), then multiply.
                        rc1 = den_pool.tile([1, QHW], F32, name="rc1", tag="rc1")
                        nc.vector.reciprocal(rc1[:, :], ot[64:65, :])
                        rb = psS.tile([64, QHW], F32, tag="psS")
                        for nch in range(2):
                            nc.tensor.matmul(
                                rb[:, nch * 512:(nch + 1) * 512], ones64[:, :],
                                rc1[:, nch * 512:(nch + 1) * 512],
                                start=True, stop=True)
                        rec = den_pool.tile([64, QHW], F32, name="rec", tag="rec")
                        nc.vector.tensor_copy(rec[:, :], rb[:, :])
                        nc.vector.tensor_tensor(
                            opairs[pr_i][hh * 64:(hh + 1) * 64, :],
                            ot[0:64, :], rec[:, :], op=ALU.mult)

                    # output projection for this q half
                    for q_t in range(QHW // 128):
                        for nch in range(2):
                            ps = psS.tile([128, 512], F32, tag="psS")
                            for pr_i in range(2):
                                nc.tensor.matmul(
                                    ps[:, :],
                                    opairs[pr_i][:, q_t * 128:(q_t + 1) * 128],
                                    wo_sb[pr_i][:, nch * 512:(nch + 1) * 512],
                                    start=(pr_i == 0), stop=(pr_i == 1))
                            ost = ostage_pool.tile([128, 512], F32, name="ost",
                                                   tag="ost")
                            nc.vector.tensor_copy(ost[:, :], ps[:, :])
                            r0 = qh * QHW + q_t * 128
                            nc.sync.dma_start(
                                out=out_d.ap()[r0:r0 + 128,
                                               nch * 512:(nch + 1) * 512],
                                in_=ost[:, :])

    nc.compile()
    return nc


def _get_program(use_attn_mask: bool):
    key = (use_attn_mask,)
    if key not in _PROGRAM_CACHE:
        _PROGRAM_CACHE[key] = _build_program(use_attn_mask)
    return _PROGRAM_CACHE[key]


def _prep_in_maps(query, key, value, key_padding_mask, attn_mask, stoich_frac,
                  Wq, bq, Wk, bk, Wv, bv, Wo, stoich_alpha, use_attn_mask):
    bf = ml_dtypes.bfloat16
    f16 = np.float16
    alpha = float(stoich_alpha)
    # identity scale: 8*alpha normally; when alpha == 0 the stoich term is
    # removed by zeroing f instead, so the identity keeps scale 8 for the
    # (optional) attn_mask path.
    if alpha != 0.0:
        id_scale, am_scale = 8.0 * alpha, 1.0 / alpha
    else:
        id_scale, am_scale = 8.0, 1.0
        stoich_frac = np.zeros_like(stoich_frac)
    alpha2 = np.array([id_scale, 0.0], np.float32)
    xt = {}
    for b in range(B):
        xt["q", b] = np.ascontiguousarray(query[b].T).astype(bf)
        xt["k", b] = np.ascontiguousarray(key[b].T).astype(bf)
        xt["v", b] = np.ascontiguousarray(value[b].T).astype(bf)
    wqT = np.ascontiguousarray(Wq.T).astype(bf)
    wkT = np.ascontiguousarray(Wk.T).astype(bf)
    wvT = np.ascontiguousarray(Wv.T).astype(bf)
    if use_attn_mask:
        # pre-divided by alpha: the scaled identity multiplies it back
        am8t = np.ascontiguousarray(am_scale * attn_mask.T).astype(f16)
    in_maps = []
    for c in range(N_CORES):
        b = c // 4
        g = c % 4
        sl = slice(g * DSL, (g + 1) * DSL)
        m = {
            "xq_t": xt["q", b],
            "xk_t": xt["k", b],
            "xv_t": xt["v", b],
            "wq_t": np.ascontiguousarray(wqT[:, sl]),
            "wk_t": np.ascontiguousarray(wkT[:, sl]),
            "wv_t": np.ascontiguousarray(wvT[:, sl]),
            "wo_t": np.ascontiguousarray(Wo[:, sl].T).astype(bf),
            "bq_d": np.ascontiguousarray(bq[sl]).astype(np.float32),
            "bk_d": np.ascontiguousarray(bk[sl]).astype(np.float32),
            "bv_d": np.ascontiguousarray(bv[sl]).astype(np.float32),
            "f_d": np.ascontiguousarray(stoich_frac[b]).astype(np.float32),
            "kpm_d": np.ascontiguousarray(key_padding_mask[b]).astype(np.float32),
            "alpha_d": alpha2,
        }
        if use_attn_mask:
            m["am_d"] = am8t
        in_maps.append(m)
    return in_maps


def kernel(query, key, value, key_padding_mask, attn_mask, stoich_frac,
           Wq, bq, Wk, bk, Wv, bv, Wo, bo, stoich_alpha):
    query = np.asarray(query, np.float32)
    key = np.asarray(key, np.float32)
    value = np.asarray(value, np.float32)
    key_padding_mask = np.asarray(key_padding_mask)
    attn_mask = np.asarray(attn_mask, np.float32)
    stoich_frac = np.asarray(stoich_frac, np.float32)
    use_attn_mask = bool(np.any(attn_mask))

    nc = _get_program(use_attn_mask)
    in_maps = _prep_in_maps(query, key, value, key_padding_mask, attn_mask,
                            stoich_frac, Wq, bq, Wk, bk, Wv, bv, Wo,
                            stoich_alpha, use_attn_mask)
    res = run_bass_kernel_spmd(nc, in_maps, core_ids=list(range(N_CORES)))

    out = np.zeros((B, T, D_MODEL), np.float32)
    for c in range(N_CORES):
        out[c // 4] += res.results[c]["out_d"]
    out += np.asarray(bo, np.float32)[None, None, :]
    return out
